# revision 27
# baseline (speedup 1.0000x reference)
"""Multi-head attention forward (B=2, S=2048, D=1024, H=16) on 8 TRN2 cores.

Sharding: hybrid tensor/data parallel. Cores 0-3 take batch 0, cores 4-7
batch 1; within a batch each core owns 4 heads (256 of 1024 features).
The host pre-transposes activations/weights and sums the 4 partial output
projections per batch (+ output bias) at the end.

Schedule: the kernel is ACT-bound (exp of 16.8M scores/core ~= 147us at
1 elem/cycle/lane), so everything else hides under the attention k-loop:
  - q/k projected feature-on-partition (qT/kT = W @ X.T), v projected in
    natural token-on-partition orientation straight into vaug (64 v cols
    + ones column per head for the softmax denominator row).
  - scores sT = kT.T @ qT, two heads row-packed in the PE (K=64 tiles at
    row groups 0/64 run concurrently); exp on ACT; PV skewed behind exp.
  - projections, output projection, and normalization broadcasts are
    closures popped between score matmuls as PE filler, ordered by
    deadline, so the PE stream never head-of-line blocks.
  - softmax normalization: denominators from the ones-column row of the
    PV psum, reciprocal_approx_fast on DVE, broadcast via two col-packed
    K=1 matmuls, applied by DVE multiplies one iteration later.
"""

import sys
import types
from collections import deque

import numpy as np

# ---------------------------------------------------------------------------
# Problem constants (hardcoded; kernel.py must be self-contained)
# ---------------------------------------------------------------------------
B = 2  # batch
S = 2048  # sequence length
D = 1024  # model dim
H = 16  # heads
DK = D // H  # 64 head dim
NCORES = 8
CPB = NCORES // B  # cores per batch = 4
FH = D // CPB  # features per core = 256 (4 heads)
P = 128
KD = D // P  # 8 contraction k-tiles for projections
KT = S // P  # 16 key-token tiles
NM = FH // P  # 2 m-tiles per core = head pairs
QS = 512  # q-slice width for the attention inner loop
NQS = S // QS  # 4
NC = 4  # x-chunk columns count (512 wide each)
NEG_SCALE = 1.0 / np.sqrt(DK)  # folded into Wq/bq on the host


def _install_ntff_hook():
    """Recreate antenv.axon_hooks so trace=True can profile via axon."""
    if "antenv.axon_hooks" in sys.modules:
        return
    try:
        import antenv
    except ImportError:
        return
    mod = types.ModuleType("antenv.axon_hooks")
    mod._hook = None
    mod.set_axon_ntff_profile_hook = lambda h: setattr(mod, "_hook", h)
    mod.get_axon_ntff_profile_hook = lambda: mod._hook
    sys.modules["antenv.axon_hooks"] = mod
    antenv.axon_hooks = mod
    try:
        from trn_agent_boot.trn_boot import _ntff_profile_via_ctypes

        mod.set_axon_ntff_profile_hook(
            _ntff_profile_via_ctypes("/opt/axon/libaxon_pjrt.so")
        )
    except Exception:
        pass


_NC_CACHE = {}


def _build_nc():
    """Build the per-core Bass program (identical on all 8 cores)."""
    from contextlib import ExitStack

    import concourse.bass as bass  # noqa: F401
    import concourse.mybir as mybir
    import concourse.tile as tile
    from concourse import bacc

    f32 = mybir.dt.float32
    f16 = mybir.dt.float16
    AF = mybir.ActivationFunctionType

    nc = bacc.Bacc()

    xtq = nc.dram_tensor("xtq", [D, S], f16, kind="ExternalInput")
    xtk = nc.dram_tensor("xtk", [D, S], f16, kind="ExternalInput")
    xtv = nc.dram_tensor("xtv", [D, S], f16, kind="ExternalInput")
    wqt = nc.dram_tensor("wqt", [D, FH], f16, kind="ExternalInput")
    wkt = nc.dram_tensor("wkt", [D, FH], f16, kind="ExternalInput")
    wvt = nc.dram_tensor("wvt", [D, FH], f16, kind="ExternalInput")
    wot = nc.dram_tensor("wot", [FH, D], f16, kind="ExternalInput")
    bqd = nc.dram_tensor("bqd", [P, NM], f32, kind="ExternalInput")
    bkd = nc.dram_tensor("bkd", [P, NM], f32, kind="ExternalInput")
    bvd = nc.dram_tensor("bvd", [P, NM], f32, kind="ExternalInput")
    out = nc.dram_tensor("out", [S, D], f16, kind="ExternalOutput")

    with tile.TileContext(nc) as tc, ExitStack() as ctx:
        const = ctx.enter_context(tc.tile_pool(name="const", bufs=1))
        wpool = ctx.enter_context(tc.tile_pool(name="wpool", bufs=1))
        persist = ctx.enter_context(tc.tile_pool(name="persist", bufs=1))
        xpool = ctx.enter_context(tc.tile_pool(name="xpool", bufs=16))
        expool = ctx.enter_context(tc.tile_pool(name="expool", bufs=8))
        npool = ctx.enter_context(tc.tile_pool(name="npool", bufs=2))
        obpool = ctx.enter_context(tc.tile_pool(name="obpool", bufs=2))

        # --- constants (cheap, before the big DMAs) ---
        ones_f32 = const.tile([P, DK], f32)
        nc.vector.memset(ones_f32, 1.0)
        ones1 = const.tile([1, DK], f16)
        nc.vector.tensor_copy(ones1, ones_f32[0:1, :])
        bq_sb = const.tile([P, NM], f32)
        bk_sb = const.tile([P, NM], f32)
        bv_sb = const.tile([P, NM], f32)
        nc.sync.dma_start(bq_sb, bqd[:, :])
        nc.sync.dma_start(bk_sb, bkd[:, :])
        nc.sync.dma_start(bv_sb, bvd[:, :])

        # --- persistent activations ---
        qt_sb = persist.tile([P, NM, S], f16)
        kt_sb = persist.tile([P, NM, S], f16)
        vt_sb = persist.tile([P, NM, S], f16)
        vaug_sb = persist.tile([P, KT, 4 * P], f16)
        ctx_sb = persist.tile([P, NM, S], f16)
        vaug4 = vaug_sb.rearrange("p t (h x) -> p t h x", x=P)
        nc.vector.memset(vaug_sb, 0.0)
        nc.vector.tensor_copy(
            vaug4[:, :, :, DK : DK + 1],
            ones_f32.rearrange("p (t h x) -> p t h x", h=4, x=1),
        )

        # --- weights: DMA'd in consumption order interleaved with x chunks
        wq_sb = wpool.tile([P, KD, FH], f16)
        wk_sb = wpool.tile([P, KD, FH], f16)
        wv_sb = wpool.tile([P, KD, FH], f16)
        wo_sb = wpool.tile([P, NM, D], f16)

        # x chunk tiles: xq/xk/xv each as (ko, c) tiles of [P, QS]
        xq_t = [[None] * NC for _ in range(KD)]
        xk_t = [[None] * NC for _ in range(KD)]
        xv_t = [[None] * NC for _ in range(KD)]

        def issue_x(tag, store, xdram, c):
            for ko in range(KD):
                t = xpool.tile(
                    [P, QS], f16, tag=tag, name=f"{tag}{ko}_{c}", bufs=4 * KD
                )
                nc.sync.dma_start(
                    t, xdram[ko * P : (ko + 1) * P, c * QS : (c + 1) * QS]
                )
                store[ko][c] = t

        # All input DMAs pre-issued in consumption order (rings hold all 4
        # column-chunks per tensor, so no WAR hazards).
        nc.sync.dma_start(wq_sb, wqt[:, :].rearrange("(ko p) f -> p ko f", p=P))
        issue_x("xq", xq_t, xtq, 0)
        nc.sync.dma_start(wk_sb, wkt[:, :].rearrange("(ko p) f -> p ko f", p=P))
        issue_x("xk", xk_t, xtk, 0)
        nc.sync.dma_start(wv_sb, wvt[:, :].rearrange("(ko p) f -> p ko f", p=P))
        issue_x("xv", xv_t, xtv, 0)
        issue_x("xk", xk_t, xtk, 1)
        issue_x("xv", xv_t, xtv, 1)
        issue_x("xk", xk_t, xtk, 2)
        issue_x("xv", xv_t, xtv, 2)
        issue_x("xk", xk_t, xtk, 3)
        issue_x("xv", xv_t, xtv, 3)
        issue_x("xq", xq_t, xtq, 1)
        issue_x("xq", xq_t, xtq, 2)
        issue_x("xq", xq_t, xtq, 3)
        nc.sync.dma_start(wo_sb, wot[:, :].rearrange("(m p) d -> p m d", p=P))

        # ------------------------------------------------------------------
        # PSUM pools: sc ring2 (2 banks each) + cx ring2 (1 bank each)
        # + shared mm ring2 (1 bank each) = exactly 8 banks
        # ------------------------------------------------------------------
        ps_sc = ctx.enter_context(tc.tile_pool(name="ps_sc", bufs=2, space="PSUM"))
        ps_cx = ctx.enter_context(tc.tile_pool(name="ps_cx", bufs=2, space="PSUM"))
        ps_mm = ctx.enter_context(tc.tile_pool(name="ps_mm", bufs=2, space="PSUM"))

        # --- PE filler closures -------------------------------------------
        def qk_proj(w_sb, x_t, b_sb, dst, c, m):
            def run():
                ps = ps_mm.tile([P, QS], f32, tag="mm", name=f"pj{id(w_sb)}_{c}_{m}")
                for ko in range(KD):
                    nc.tensor.matmul(
                        ps,
                        lhsT=w_sb[:, ko, m * P : (m + 1) * P],
                        rhs=x_t[ko][c],
                        start=(ko == 0),
                        stop=(ko == KD - 1),
                    )
                nc.vector.tensor_scalar_add(
                    dst[:, m, c * QS : (c + 1) * QS], ps, b_sb[:, m : m + 1]
                )

            return run

        def vt_proj(c, m):
            base = qk_proj(wv_sb, xv_t, bv_sb, vt_sb, c, m)

            def run():
                base()
                # transpose each 128-token tile into natural-layout vaug via
                # a contiguous staging tile (xbar DMA needs 2D dst), then a
                # strided DVE copy into the per-head blocks
                for kt in range(4 * c, 4 * c + 4):
                    stg = npool.tile(
                        [P, P], f16, tag="vstg", name=f"vs{kt}_{m}", bufs=4
                    )
                    nc.sync.dma_start_transpose(
                        stg, vt_sb[:, m, kt * P : (kt + 1) * P]
                    )
                    nc.vector.tensor_copy(
                        vaug4[:, kt, 2 * m : 2 * m + 2, 0:DK],
                        stg.rearrange("p (h x) -> p h x", x=DK),
                    )

            return run

        def norm_tail(pair, qs, cxs, rc2):
            q0 = qs * QS

            def run():
                bc = ps_mm.tile([P, QS], f32, tag="mm", name=f"bc{pair}_{qs}")
                nc.tensor.matmul(
                    bc[0:DK, :], lhsT=ones1, rhs=rc2[0], start=True, stop=True
                )
                nc.tensor.matmul(
                    bc[DK : 2 * DK, :],
                    lhsT=ones1,
                    rhs=rc2[1],
                    start=True,
                    stop=True,
                    tile_position=(0, DK),
                )
                for h in range(2):
                    nc.vector.tensor_mul(
                        ctx_sb[DK * h : DK * (h + 1), pair, q0 : q0 + QS],
                        cxs[h],
                        bc[DK * h : DK * (h + 1), :],
                    )

            return run

        def out_proj(mt):
            def run():
                for ns in range(D // QS):
                    ops = ps_mm.tile([P, QS], f32, tag="mm", name=f"op{mt}_{ns}")
                    for pair in range(NM):
                        nc.tensor.matmul(
                            ops,
                            lhsT=ctx_sb[:, pair, mt * P : (mt + 1) * P],
                            rhs=wo_sb[:, pair, ns * QS : (ns + 1) * QS],
                            start=(pair == 0),
                            stop=(pair == NM - 1),
                        )
                    ob = obpool.tile([P, QS], f16, tag="ob", name=f"ob{mt}_{ns}")
                    nc.vector.tensor_copy(ob, ops)
                    nc.sync.dma_start(
                        out[mt * P : (mt + 1) * P, ns * QS : (ns + 1) * QS], ob
                    )

            return run

        pe_q = deque()

        def pop_pe(n):
            for _ in range(n):
                if pe_q:
                    pe_q.popleft()()

        # --- pre-phase: minimum work to start (qs0, pair0) scores ---------
        qk_proj(wq_sb, xq_t, bq_sb, qt_sb, 0, 0)()
        qk_proj(wk_sb, xk_t, bk_sb, kt_sb, 0, 0)()

        # iter 0 fillers, deadline-ordered: v chunks pace the PV loop,
        # k chunks pace the score loop.
        pe_q.extend(
            [
                vt_proj(0, 0),
                vt_proj(0, 1),
                qk_proj(wk_sb, xk_t, bk_sb, kt_sb, 1, 0),
                vt_proj(1, 0),
                vt_proj(1, 1),
                qk_proj(wk_sb, xk_t, bk_sb, kt_sb, 2, 0),
                vt_proj(2, 0),
                vt_proj(2, 1),
                qk_proj(wk_sb, xk_t, bk_sb, kt_sb, 3, 0),
                vt_proj(3, 0),
                vt_proj(3, 1),
                qk_proj(wk_sb, xk_t, bk_sb, kt_sb, 0, 1),
                qk_proj(wq_sb, xq_t, bq_sb, qt_sb, 0, 1),
            ]
        )

        # ------------------------------------------------------------------
        # Attention iterations: (qs outer, pair inner)
        # ------------------------------------------------------------------
        for it in range(NQS * NM):
            qs, pair = divmod(it, NM)
            q0 = qs * QS
            first = it <= 1
            skew = 7 if it == 0 else 2
            # push this iteration's new filler work (deadline order)
            if it == 1:
                pe_q.extend(
                    [
                        qk_proj(wk_sb, xk_t, bk_sb, kt_sb, 1, 1),
                        qk_proj(wk_sb, xk_t, bk_sb, kt_sb, 2, 1),
                        qk_proj(wk_sb, xk_t, bk_sb, kt_sb, 3, 1),
                        qk_proj(wq_sb, xq_t, bq_sb, qt_sb, 1, 0),
                    ]
                )
            elif it == 2:
                pe_q.append(qk_proj(wq_sb, xq_t, bq_sb, qt_sb, 1, 1))
            elif it == 3:
                pe_q.append(qk_proj(wq_sb, xq_t, bq_sb, qt_sb, 2, 0))
            elif it == 4:
                pe_q.append(qk_proj(wq_sb, xq_t, bq_sb, qt_sb, 2, 1))
            elif it == 5:
                pe_q.append(qk_proj(wq_sb, xq_t, bq_sb, qt_sb, 3, 0))
            elif it == 6:
                pe_q.append(qk_proj(wq_sb, xq_t, bq_sb, qt_sb, 3, 1))

            cx = [
                ps_cx.tile([P, QS], f32, tag="cx", name=f"cx{it}_{h}")
                for h in range(2)
            ]
            exq = []
            for kt in range(KT):
                sc = ps_sc.tile([P, 2 * QS], f32, tag="sc", name=f"sc{it}_{kt}")
                for h in range(2):
                    nc.tensor.matmul(
                        sc[:, h * QS : (h + 1) * QS],
                        lhsT=kt_sb[
                            DK * h : DK * (h + 1), pair, kt * P : (kt + 1) * P
                        ],
                        rhs=qt_sb[DK * h : DK * (h + 1), pair, q0 : q0 + QS],
                        start=True,
                        stop=True,
                    )
                ex = expool.tile([P, 2 * QS], f16, tag="ex")
                nc.scalar.activation(ex, sc, AF.Exp)
                exq.append((kt, ex))
                if first:
                    pop_pe(2)
                elif kt % 2 == 1 and kt >= 3:
                    pop_pe(1)
                if len(exq) > skew:
                    pv_kt, pv_ex = exq.pop(0)
                    for h in range(2):
                        nc.tensor.matmul(
                            cx[h],
                            lhsT=vaug4[:, pv_kt, 2 * pair + h, :],
                            rhs=pv_ex[:, h * QS : (h + 1) * QS],
                            start=(pv_kt == 0),
                            stop=(pv_kt == KT - 1),
                        )
            for pv_kt, pv_ex in exq:
                for h in range(2):
                    nc.tensor.matmul(
                        cx[h],
                        lhsT=vaug4[:, pv_kt, 2 * pair + h, :],
                        rhs=pv_ex[:, h * QS : (h + 1) * QS],
                        start=(pv_kt == 0),
                        stop=(pv_kt == KT - 1),
                    )

            # norm prep (DVE only, off the PE stream): ctx rows + denominator
            # row to SBUF, fast reciprocal, f16 broadcast sources
            cxs = []
            rc2 = []
            for h in range(2):
                cs = npool.tile([DK, QS], f16, tag="cxs", name=f"cxs{it}_{h}", bufs=4)
                nc.vector.tensor_copy(cs, cx[h][0:DK, :])
                cxs.append(cs)
                dh = npool.tile([1, QS], f32, tag="dh", name=f"dh{it}_{h}", bufs=2)
                nc.vector.tensor_copy(dh, cx[h][DK : DK + 1, :])
                rcf = npool.tile([1, QS], f32, tag="rcf", name=f"rcf{it}_{h}", bufs=2)
                nc.vector.reciprocal_approx_fast(rcf, dh)
                rc = npool.tile([1, QS], f16, tag="rc2", name=f"rc2_{it}_{h}", bufs=4)
                nc.vector.tensor_copy(rc, rcf)
                rc2.append(rc)
            pe_q.append(norm_tail(pair, qs, cxs, rc2))
            if pair == 1:
                for sub in range(QS // P):
                    pe_q.append(out_proj(qs * (QS // P) + sub))

        # drain the remaining deferred work (last norm + last out-proj)
        while pe_q:
            pop_pe(1)

    nc.finalize()
    return nc


def _get_nc():
    if "nc" not in _NC_CACHE:
        _install_ntff_hook()
        _NC_CACHE["nc"] = _build_nc()
    return _NC_CACHE["nc"]


def _make_in_maps(query, key, value, Wq, bq, Wk, bk, Wv, bv, Wo):
    qn = np.asarray(query, np.float32)
    kn = np.asarray(key, np.float32)
    vn = np.asarray(value, np.float32)
    Wq = np.asarray(Wq, np.float32)
    Wk = np.asarray(Wk, np.float32)
    Wv = np.asarray(Wv, np.float32)
    Wo = np.asarray(Wo, np.float32)
    bq = np.asarray(bq, np.float32)
    bk = np.asarray(bk, np.float32)
    bv = np.asarray(bv, np.float32)

    xt = {}
    for b in range(B):
        xt[b] = (
            np.ascontiguousarray(qn[b].T).astype(np.float16),
            np.ascontiguousarray(kn[b].T).astype(np.float16),
            np.ascontiguousarray(vn[b].T).astype(np.float16),
        )

    in_maps = []
    for c in range(NCORES):
        b, hp = divmod(c, CPB)
        sl = slice(hp * FH, (hp + 1) * FH)
        in_maps.append(
            {
                "xtq": xt[b][0],
                "xtk": xt[b][1],
                "xtv": xt[b][2],
                "wqt": np.ascontiguousarray((Wq[sl, :] * NEG_SCALE).T).astype(
                    np.float16
                ),
                "wkt": np.ascontiguousarray(Wk[sl, :].T).astype(np.float16),
                "wvt": np.ascontiguousarray(Wv[sl, :].T).astype(np.float16),
                "wot": np.ascontiguousarray(Wo[:, sl].T).astype(np.float16),
                "bqd": np.ascontiguousarray(
                    (bq[sl] * NEG_SCALE).reshape(NM, P).T
                ),
                "bkd": np.ascontiguousarray(bk[sl].reshape(NM, P).T),
                "bvd": np.ascontiguousarray(bv[sl].reshape(NM, P).T),
            }
        )
    return in_maps


def _run(inputs, trace=False):
    from concourse.bass_utils import run_bass_kernel_spmd

    nc = _get_nc()
    in_maps = _make_in_maps(
        inputs["query"],
        inputs["key"],
        inputs["value"],
        inputs["Wq"],
        inputs["bq"],
        inputs["Wk"],
        inputs["bk"],
        inputs["Wv"],
        inputs["bv"],
        inputs["Wo"],
    )
    res = run_bass_kernel_spmd(nc, in_maps, list(range(NCORES)), trace=trace)
    bo = np.asarray(inputs["bo"], np.float32)
    out = np.zeros((B, S, D), np.float32)
    for c in range(NCORES):
        out[c // CPB] += res.results[c]["out"].astype(np.float32)
    out += bo[None, None, :]
    return out, res


def kernel(**inputs) -> np.ndarray:
    out, _ = _run(inputs, trace=False)
    return out
